# revision 47
# baseline (speedup 1.0000x reference)
"""Trainium2 Bass kernel for nn_MultiHeadAttention_30846455119878.

8-core strategy:
  - Attention phase is head-sharded: core m owns heads {2m, 2m+1}. Every core
    computes q/k/v projections for its 2 heads over all B*T tokens, then causal
    softmax attention per (batch, head).
  - The output projection contracts over ALL heads, so instead of an expensive
    AllReduce of [B,T,C] partials, each batch's attention output features
    ([128 feats x T]) are exchanged with a small AllToAll that re-shards from
    heads -> tokens. Each core then computes the full output projection for its
    1/8 token slice (contraction over all 1024 features) plus bias, locally.
  - Host side: x is passed pre-transposed as x^T [C, B*T] in bf16 (fp32 can't
    DMA-transpose on TRN2); all matmuls run bf16 x bf16 -> fp32 PSUM.

Layouts (per core):
  xt   [128, NCH, CK, TCH] bf16  xt[p, ch, o, t] = x[ch*512+t, o*128+p] (chunk-contiguous)
  x8   same layout, fp8 e4m3 (feeds q/k DoubleRow projections)
  wq/wk [128, CK, 128] fp8 e4m3, host-scaled x32; wv [128, CK, 128] bf16
  wo   [128, 8, C] bf16     wo[p, j, c] = Wo[j*128+p, c]
  bo   [1, C] bf16
  mask [128, 4, 512] bf16   mask[p, j, t] = 1 if t >= p + j*128 else 0  (causal diag blocks)
  out  [B, T/8, C] bf16     core m holds tokens [m*T/8, (m+1)*T/8) of every batch

Performance state (HW, 8 axon trn2 cores): 441,873 / 446,277 ns after a 3-min
cool-down, rel err 6.58e-3. THERMAL NOISE DOMINATES A/B TESTS: the SW/GPIO
throttlers (200us loop, ham type-1 k4/n8 = half clock) react to recent board
power; back-to-back benchmark runs degrade to 470-520us with type-1 active
300us+. Benchmark protocol: sleep 180 first, trust only same-temperature
pairs. 2026-08-10 session wins (all kept in this file):
  - xt prefetched a full batch ahead (issue after phase A of prev batch);
    per-chunk-contiguous DRAM layout [128, NCH, CK, TCH]. The old just-in-time
    fetch starved 20-45us during a2a windows and each PE stall re-throttled
    the clock (one 28us gap cost ~40us extra of half-clock).
  - finalize (rb-broadcast + normalize + staging + cc) DEFERRED into the next
    batch's phase-A window: kills all 4 batch-boundary PE gaps (7.2us each,
    the den->recip->rb chain), and the a2as collapse 47->15-23us because
    staging rides the quiet phase-A DMA window.
  - q/k projections in fp8 e4m3 DoubleRow (wq/wk host-scaled x32, exp scale
    /1024): -16k PE cols/batch, rel err 4.0e-3 -> 6.6e-3. WORKS on HW.
  - dummy 16-elem AllToAll at boot: absorbs the 11.5us cc first-trigger
    latency AND pre-aligns cores (real a2as then ~20us).
  - rcv DMA on gpsimd directly after its cc (the documented-safe spot);
    rcvpool bufs=4 REQUIRED (bufs=2 deadlocks cc3 behind phase-C-b0 WAR).
  - bias folded into a boot-time PE broadcast + DVE tensor_add in phase C
    (PSUM input); out/osb bf16 (host upcasts); staging merged 16->2 DMAs/batch
    via DRAM-side rearrange; den tiles bf16.
FAILED this session (do not retry naively):
  - fp8 DoubleRow SCORES: needs q/k in [32,2,h,T] -> 32-partition PSUM->SBUF
    copies; narrow copies are free-dim-bound (cost = full-width) on DVE AND
    ScalarE -> +60-100us whichever engine takes them. Cross-quadrant
    32-partition copies themselves ARE correct on both engines (HW-verified).
  - v projection in fp8 e4m3: rel err 2.95e-2 > 2e-2 gate (v errors pass
    linearly to the output; softmax absorbs q/k errors ~10x better).
  - mask in fp8: DVE tensor_mul bf16xfp8 is ~1.6x slower per element.
  - phase C accumulators on psB: 470us vs 445 (single warm run, weak signal).
  - bulk chunk DMAs on the gpsimd queue: software-driven, ~25us for 512KB
    under boot contention (vs ~2us hardware queues) -> two new 7us PE gaps.
    gpsimd DMAs are fine ONLY for small/slack loads (wv, mask, rcv).
  - boot xt chunks split across scalar/sync: neutral (boot is ~8us DMA
    cold-start dominated, not queue-depth dominated).
PE is 90-97% busy (interval-merged) inside every batch window at the granted
clock -> cross-batch interleave of phase A into phase B has ~nothing to win;
per-tcb recip for the last batch is zero-sum (rb MMs are PE-serial either
way). Tail a2a wait (~22-47us) has no deferrable work left to hide it.
Current trace shape (cool run): batches ~80us (PE-column-bound at whatever
clock the throttler grants; ScalarE exp 50us/batch paces phase B only when
fully warm), boot ~12us, tail ~45us = trigger3 + a2a3 20us + phase C b3 12us
+ drain 11us. PE cols/batch ~107k = 44.6us floor at 2.4GHz.

Tuning results (HW-measured, for future iteration):
  - Batched 128-lane reciprocal via SBUF->SBUF reshape DMA: 579 -> 468us. KEEP.
  - Diag-block column skip (c0 slicing): 613 -> 579us. KEEP.
  - v computed directly in [s,d] layout (no DMA-transpose round trip): KEEP.
  - psA/psB = 5/3 best; 6/2 -> 525us; 4/4 untested alone.
  - qk/v1 bufs 3, epool 8, rb-copy on DVE: all within +-4us run noise.
  - Per-half AllToAll split (8 collectives): 564us — entry-barrier floor dominates.
  - Outproj inlined into the b-loop after each collective: 451us but WRONG results
    (rel 0.86) — HW-only ordering hazard, unresolved; sim passes. Worth debugging
    with dbg taps on rcv: the ~17us speedup is real if made sound.
  - reciprocal_approx_fast / gpsimd partition_broadcast: BROKEN in this runtime
    (custom DVE/GPSIMD library ops garbage through the bass2jax compile path).
  - XBAR dma_start_transpose: silently shears with strided src / unaligned dst.
  - This walrus build: max 1 sync-wait per instruction -> must use bacc.Bacc.
  - bf16 AllToAll + TensorE tile_position row-packing: verified correct on HW.

2026-08-09 session (all HW-measured, 2+ runs each) - failed restructures; the
pipeline is a tight ScalarE<->PE equilibrium (both ~97% busy per 86us batch);
ANY extra dependency injected inside the batch loop breaks HAM warmth (PE
drops to 1.2GHz; throttle_active 90us -> 500us) and costs +80..120us:
  - v as [F,T] proj + PE-transpose to [s,d]: -20us PE on paper, measured +85us
    (DVE<->PE ping-pong in phase A delays v1, stalls phase B).
  - Per-tcb normalize+staging: collectives drop to 9-16us (vs 26-44) but wall
    +115us (normalize chain mid-phase-B breaks the score->exp->att pipeline).
  - Exp batched to [128,1024]/[128,2048] ACTs (ScalarE 58->40us/batch): forces
    PSUM score bufs 2 (vs 5) -> scores wait ACT latency every group -> micro-
    stalls -> cold PE. Net +80us. 8 PSUM banks cannot hold wider score groups
    plus 2 att banks plus proj/rb rotation; tag-sharing couples WARs, worse.
  - Boot PE-warmup spin (40 MMs): neutral. Tail keep-warm spin: +5us.
  - collective_compute BLOCKS its engine queue until completion; Tile REORDERS
    engine streams (a DMA sync-dep'd on a collective was hoisted mid-batch on
    the Scalar queue, head-of-line blocking ACTs 35us). Anything depending on
    a collective must ride gpsimd AFTER its collective or sit at program end.
  - SBUF-side AP rearrange("p (j t) -> j p t") in a DMA reads garbage -> NaN.
  - GPSIMD cannot touch PSUM; DVE TensorTensor allows only ONE PSUM input;
    engine PSUM reads must start at 32-aligned partitions; PE transpose output
    dtype must equal input dtype (bf16 PSUM tile).
  - A2A duration is mostly entry skew: boot-stagger across the 8 axon cores
    (init barrier 38-102us observed) shifts everything; benchmark twice.
"""

import sys

if "/opt/trn_rl_repo" not in sys.path:
    sys.path.insert(0, "/opt/trn_rl_repo")

import numpy as np
import ml_dtypes

import concourse.bass as bass
import concourse.tile as tile
from concourse import bacc, mybir
from concourse.bass_utils import run_bass_kernel_spmd
from concourse.tile_rust import add_dep_helper

BF16 = ml_dtypes.bfloat16

# Full problem dims
B_FULL, T_FULL, C_FULL, H_FULL, D_HEAD = 4, 2048, 1024, 16, 64
N_CORES = 8
HPC = H_FULL // N_CORES  # heads per core = 2
F = HPC * D_HEAD         # per-core attention feature rows = 128
TCH = 512                # query-chunk (free dim of score matmuls)
D = D_HEAD


def build_nc(B=B_FULL, T=T_FULL, C=C_FULL, debug=False):
    """Build the SPMD Bass graph (same graph on all 8 cores)."""
    dt = mybir.dt
    CK = C // 128        # contraction chunks for projections
    NTC = T // TCH       # query chunks per sequence
    NSB = T // 128       # key blocks per sequence
    SBB = TCH // 128     # key blocks that overlap one query chunk diagonal = 4
    TS = (B * T) // (B * N_CORES)  # token shard per (batch, core) = T // 8
    CO = H_FULL * D_HEAD  # output feature dim (Wo cols) = 1024
    TT = 128 if TS % 128 == 0 else TS  # token tile for output projection
    # wq/wk are host-scaled by 32 each (fp8 range); undo the 1024x here
    scale = float(1.0 / np.sqrt(C)) / 1024.0

    nc = bacc.Bacc()
    NCH = (B * T) // TCH  # token chunks over the whole input
    xt_d = nc.declare_dram_parameter("xt", [128, NCH, CK, TCH], dt.bfloat16, isOutput=False)
    # q/k projections run fp8 DoubleRow (2 contraction chunks per pass, half
    # the PE columns). Host pre-scales Wq/Wk by 32 to dodge e4m3 subnormals;
    # undone in the exp scale. v stays bf16: e4m3 there costs 2.9e-2 rel err
    # (values are linearly exposed in the output).
    x8_d = nc.declare_dram_parameter("x8", [128, NCH, CK, TCH], dt.float8e4, isOutput=False)
    wq_d = nc.declare_dram_parameter("wq", [128, CK, F], dt.float8e4, isOutput=False)
    wk_d = nc.declare_dram_parameter("wk", [128, CK, F], dt.float8e4, isOutput=False)
    wv_d = nc.declare_dram_parameter("wv", [128, CK, F], dt.bfloat16, isOutput=False)
    wo_d = nc.declare_dram_parameter("wo", [128, N_CORES, CO], dt.bfloat16, isOutput=False)
    bo_d = nc.declare_dram_parameter("bo", [1, CO], dt.bfloat16, isOutput=False)
    mask_d = nc.declare_dram_parameter("mask", [128, SBB, TCH], dt.bfloat16, isOutput=False)
    out_d = nc.declare_dram_parameter("out", [B, TS, CO], dt.bfloat16, isOutput=True)

    dbg = {}
    if debug:
        dbg["attn"] = nc.declare_dram_parameter("dbg_attn", [D, T], dt.bfloat16, isOutput=True)
        dbg["v1"] = nc.declare_dram_parameter("dbg_v1", [128, T // 128, HPC, 80], dt.bfloat16, isOutput=True)
        dbg["rcv"] = nc.declare_dram_parameter("dbg_rcv", [128, N_CORES, TS], dt.bfloat16, isOutput=True)
    dumm_i = nc.dram_tensor("cc_dummy_i", [N_CORES, 1, 16], dt.bfloat16)
    dumm_o = nc.dram_tensor("cc_dummy_o", [N_CORES, 1, 16], dt.bfloat16)
    cc_in = [nc.dram_tensor(f"cc_in{b}", [N_CORES, F, TS], dt.bfloat16) for b in range(B)]
    cc_out = [nc.dram_tensor(f"cc_out{b}", [N_CORES, F, TS], dt.bfloat16) for b in range(B)]
    rg = [list(range(N_CORES))]

    with tile.TileContext(nc) as tc:
        from contextlib import ExitStack

        with ExitStack() as ctx:
            wpool = ctx.enter_context(tc.tile_pool(name="w", bufs=1))
            xpool = ctx.enter_context(tc.tile_pool(name="xt", bufs=4))
            x8pool = ctx.enter_context(tc.tile_pool(name="x8", bufs=4))
            qkpool = ctx.enter_context(tc.tile_pool(name="qk", bufs=2))
            v1pool = ctx.enter_context(tc.tile_pool(name="v1", bufs=2))
            epool = ctx.enter_context(tc.tile_pool(name="exp", bufs=6))
            apool = ctx.enter_context(tc.tile_pool(name="attn", bufs=4))
            recpool = ctx.enter_context(tc.tile_pool(name="rec", bufs=3))
            aupool = ctx.enter_context(tc.tile_pool(name="attu", bufs=2))
            denpool = ctx.enter_context(tc.tile_pool(name="den", bufs=2))
            rcvpool = ctx.enter_context(tc.tile_pool(name="rcv", bufs=4))
            outpool = ctx.enter_context(tc.tile_pool(name="osb", bufs=2))
            psA = ctx.enter_context(tc.tile_pool(name="psA", bufs=5, space="PSUM"))
            psB = ctx.enter_context(tc.tile_pool(name="psB", bufs=3, space="PSUM"))

            # resident constants
            wq_sb = wpool.tile([128, CK, F], dt.float8e4, tag="wq")
            wk_sb = wpool.tile([128, CK, F], dt.float8e4, tag="wk")
            wv_sb = wpool.tile([128, CK, F], dt.bfloat16, tag="wv")
            wo_sb = wpool.tile([128, N_CORES, CO], dt.bfloat16, tag="wo")
            bo_sb = wpool.tile([1, CO], dt.bfloat16, tag="bo")
            mask_sb = wpool.tile([128, SBB, TCH], dt.bfloat16, tag="mask")
            ones_sb = wpool.tile([D + 1, 128], dt.bfloat16, tag="ones")
            # boot order: first q/k weights (scalar) + first xt chunks (sync)
            # so the very first projection matmul can start ASAP; everything
            # else rides other queues behind them.
            nc.scalar.dma_start(out=wq_sb, in_=wq_d[:, :, :])
            nc.scalar.dma_start(out=wk_sb, in_=wk_d[:, :, :])

            def issue_xt(b, xt_engs=None):
                # xt_engs: per-chunk queue override for the bf16 copies (boot
                # only — spreads the cold-start load over all 3 DMA queues)
                tiles = []
                for tcb in range(NTC):
                    x8_sb = x8pool.tile([128, CK, TCH], dt.float8e4, tag="x8")
                    nc.sync.dma_start(out=x8_sb, in_=x8_d[:, b * NTC + tcb, :, :])
                    xt_sb = xpool.tile([128, CK, TCH], dt.bfloat16, tag="xt")
                    eng = (xt_engs[tcb] if xt_engs and xt_engs[tcb] else nc.sync)
                    eng.dma_start(out=xt_sb, in_=xt_d[:, b * NTC + tcb, :, :])
                    tiles.append((x8_sb, xt_sb))
                return tiles

            nc.gpsimd.dma_start(out=wv_sb, in_=wv_d[:, :, :])
            # NOTE: gpsimd DMAs are software-driven and slow (~25us for 512KB
            # under boot contention) — keep bulk chunk loads off gpsimd
            xt_cur = issue_xt(0)
            nc.gpsimd.dma_start(out=mask_sb, in_=mask_d[:, :, :])
            # wo only matters at phase C; ride sync behind batch-0 x8
            nc.sync.dma_start(out=wo_sb, in_=wo_d[:, :, :])
            nc.scalar.dma_start(out=bo_sb, in_=bo_d[:, :])
            nc.vector.memset(ones_sb, 1.0)
            bias_bc = wpool.tile([128, CO], dt.bfloat16, tag="biasbc")
            # tiny warmup collective: absorbs the ~11.5us first-trigger
            # latency of the cc subsystem before the first real AllToAll
            nc.gpsimd.collective_compute(
                "AllToAll", mybir.AluOpType.bypass, replica_groups=rg,
                ins=[dumm_i.ap().opt()], outs=[dumm_o.ap().opt()],
            )

            cc_insts = []
            SLOTS = NTC * HPC

            def finalize(fb, att_un_f, rec_all_f):
                # rb-broadcast + normalize + staging + collective for batch fb.
                # Deferred into batch fb+1's phase-A window so the den->recip
                # chain never leaves PE idle at the batch boundary. Head-major
                # so head 0's staging DMA overlaps head 1's normalize (matters
                # for the last batch, where this chain is the a2a tail path).
                stg_insts = []
                attn_f = [apool.tile([D, T], dt.bfloat16, tag="attn", name=f"attn_{fb}_{hh}") for hh in range(HPC)]
                for h in range(HPC):
                    for tcb in range(NTC):
                        slot = tcb * HPC + h
                        if isinstance(rec_all_f, tuple):
                            ra_f, rz_f = rec_all_f
                            rec_src = (ra_f[0:1, slot * TCH:(slot + 1) * TCH]
                                       if slot < 6 else
                                       rz_f[0:1, (slot - 6) * TCH:(slot - 5) * TCH])
                        else:
                            rec_src = rec_all_f[0:1, slot * TCH:(slot + 1) * TCH]
                        # last batch: psB, so the rb matmuls don't queue
                        # behind psA's score-ring WAR on the lagging exps
                        rb_pool = psB if fb == B - 1 else psA
                        rb_tag = "att" if fb == B - 1 else "mm"
                        rb_ps = rb_pool.tile([D, TCH], dt.float32, tag=rb_tag)
                        nc.tensor.matmul(
                            rb_ps, lhsT=ones_sb[0:1, 0:D],
                            rhs=rec_src,
                            start=True, stop=True,
                        )
                        # one PSUM input is allowed on DVE tensor_tensor, so
                        # multiply straight out of PSUM (no rb copy)
                        nc.vector.tensor_mul(
                            attn_f[h][:, tcb * TCH:(tcb + 1) * TCH],
                            att_un_f[:, slot, :], rb_ps,
                        )
                    eng = nc.scalar if h == 0 else nc.sync
                    stg_insts.append(eng.dma_start(
                        out=cc_in[fb][:, h * D:(h + 1) * D, :].rearrange("j p t -> p j t"),
                        in_=attn_f[h],
                    ).ins)
                if debug and fb == 0:
                    nc.scalar.dma_start(out=dbg["attn"][:, :], in_=attn_f[0])
                cc = nc.gpsimd.collective_compute(
                    "AllToAll", mybir.AluOpType.bypass, replica_groups=rg,
                    ins=[cc_in[fb].ap().opt()], outs=[cc_out[fb].ap().opt()],
                )
                for s in stg_insts:
                    add_dep_helper(cc.ins, s, sync=True, reason="cc_in RAW")
                cc_insts.append(cc.ins)
                # gpsimd is blocked by the collective anyway, so a dependent
                # DMA here fires the instant the a2a lands (no head-of-line
                # risk on the busy queues)
                rcv = rcvpool.tile([128, N_CORES, TS], dt.bfloat16, tag="rcv")
                rcv_rd = nc.gpsimd.dma_start(
                    out=rcv, in_=cc_out[fb][:, :, :].rearrange("j p t -> p j t")
                )
                add_dep_helper(rcv_rd.ins, cc.ins, sync=True, reason="cc_out RAW")
                rcv_tiles.append(rcv)

            pend = None
            rcv_tiles = []
            for b in range(B):
                # ---- phase A: q/k projections ([d, t] layout) and v ([s, d] layout)
                qT = qkpool.tile([F, T], dt.bfloat16, tag="qT")
                kT = qkpool.tile([F, T], dt.bfloat16, tag="kT")
                v1 = v1pool.tile([128, NSB, HPC, 80], dt.bfloat16, tag="v1")
                nc.vector.memset(v1[:, :, :, D:D + 1], 1.0)
                for tcb in range(NTC):
                    x8_sb, xt_sb = xt_cur[tcb]
                    for w_sb, dstT in ((wq_sb, qT), (wk_sb, kT)):
                        ps = psA.tile([128, TCH], dt.float32, tag="mm")
                        for o2 in range(CK // 2):
                            nc.tensor.matmul(
                                ps,
                                lhsT=w_sb[:, 2 * o2:2 * o2 + 2, :],
                                rhs=x8_sb[:, 2 * o2:2 * o2 + 2, :],
                                start=(o2 == 0), stop=(o2 == CK // 2 - 1),
                                perf_mode=mybir.MatmulPerfMode.DoubleRow,
                            )
                        nc.vector.tensor_copy(
                            out=dstT[:, tcb * TCH:(tcb + 1) * TCH], in_=ps
                        )
                    # v directly in [s, d] layout: v[s, f] = sum_c x[s, c] Wv[c, f]
                    for ssub in range(SBB):
                        vps_full = psA.tile([128, TCH], dt.float32, tag="mm", name=f"vps_{b}_{tcb}_{ssub}")
                        vps = vps_full[:, 0:F]
                        for o in range(CK):
                            nc.tensor.matmul(
                                vps,
                                lhsT=xt_sb[:, o, ssub * 128:(ssub + 1) * 128],
                                rhs=wv_sb[:, o, :],
                                start=(o == 0), stop=(o == CK - 1),
                            )
                        st = tcb * SBB + ssub
                        for h in range(HPC):
                            nc.vector.tensor_copy(
                                out=v1[:, st, h, 0:D], in_=vps[:, h * D:(h + 1) * D]
                            )
                if b == 0:
                    # broadcast bo across 128 token rows once; phase C then
                    # adds it on DVE instead of spending a PE pass per tile
                    for c2 in range(CO // 512):
                        bps = psA.tile([128, TCH], dt.float32, tag="mm")
                        nc.tensor.matmul(
                            bps, lhsT=ones_sb[0:1, 0:128],
                            rhs=bo_sb[0:1, c2 * 512:(c2 + 1) * 512],
                            start=True, stop=True,
                        )
                        nc.vector.tensor_copy(
                            out=bias_bc[:, c2 * 512:(c2 + 1) * 512], in_=bps
                        )
                if debug and b == 0:
                    nc.scalar.dma_start(out=dbg["v1"][:, :, :, :], in_=v1)
                # prefetch the whole next batch's xt now: all of this batch's
                # chunks are consumed, so the WAR is clear, and phase B gives
                # ~60us of slack before the data is needed (the a2a traffic
                # stalls late just-in-time fetches for 20-30us otherwise).
                if b + 1 < B:
                    xt_cur = issue_xt(b + 1)
                if pend is not None:
                    finalize(*pend)
                    pend = None

                # ---- phase B: causal attention, both heads interleaved
                att_un = aupool.tile([D, SLOTS, TCH], dt.bfloat16, tag="attu")
                den_b = denpool.tile([D + 1, SLOTS * TCH], dt.bfloat16, tag="den")
                for tcb in range(NTC):
                    att_ps = [psB.tile([D + 1, TCH], dt.float32, tag="att", name=f"attps_{b}_{tcb}_{hh}") for hh in range(HPC)]
                    nsb = SBB * (tcb + 1)
                    for sb in range(nsb):
                        j0 = sb - SBB * tcb
                        # columns t < j0*128 of this (key-block, query-chunk) pair are
                        # fully causal-masked -> skip them in scores/exp/mask/att
                        c0 = j0 * 128 if j0 > 0 else 0
                        ets = []
                        for h in range(HPC):
                            s_ps = psA.tile([128, TCH], dt.float32, tag="mm")
                            nc.tensor.matmul(
                                s_ps[:, c0:TCH],
                                lhsT=kT[h * D:(h + 1) * D, sb * 128:(sb + 1) * 128],
                                rhs=qT[h * D:(h + 1) * D, tcb * TCH + c0:(tcb + 1) * TCH],
                                start=True, stop=True,
                                tile_position=(h * D, 0),
                            )
                            et = epool.tile([128, TCH], dt.bfloat16, tag="exp")
                            nc.scalar.activation(
                                out=et[:, c0:TCH], in_=s_ps[:, c0:TCH],
                                func=mybir.ActivationFunctionType.Exp, scale=scale,
                            )
                            if j0 >= 0:
                                nc.vector.tensor_mul(
                                    et[:, c0:TCH], et[:, c0:TCH],
                                    mask_sb[:, j0, c0:TCH],
                                )
                            ets.append(et)
                        for h in range(HPC):
                            nc.tensor.matmul(
                                att_ps[h][:, c0:TCH],
                                lhsT=v1[:, sb, h, 0:D + 1], rhs=ets[h][:, c0:TCH],
                                start=(sb == 0), stop=(sb == nsb - 1),
                            )
                    for h in range(HPC):
                        slot = tcb * HPC + h
                        # denominator first: it feeds the recip critical path
                        nc.vector.tensor_copy(
                            out=den_b[D:D + 1, slot * TCH:(slot + 1) * TCH],
                            in_=att_ps[h][D:D + 1, :],
                        )
                    for h in range(HPC):
                        slot = tcb * HPC + h
                        nc.vector.tensor_copy(out=att_un[:, slot, :], in_=att_ps[h][0:D, :])
                    # last batch: reciprocal for the first 3 query chunks can
                    # run now (DVE/DMA only), hiding its ~5us latency under
                    # tcb3's compute instead of exposing it pre-trigger
                    if b == B - 1 and tcb == NTC - 2:
                        den_ta = recpool.tile([128, 6 * TCH // 128], dt.bfloat16, tag="dent")
                        nc.sync.dma_start(out=den_ta, in_=den_b[D:D + 1, 0:6 * TCH])
                        rec_ta = recpool.tile([128, 6 * TCH // 128], dt.bfloat16, tag="rect")
                        with nc.allow_low_precision(reason="bf16 softmax denom recip is plenty at rel-err 2e-2"):
                            nc.vector.reciprocal(out=rec_ta, in_=den_ta)
                        rec_a = recpool.tile([1, 6 * TCH], dt.bfloat16, tag="recall")
                        nc.sync.dma_start(out=rec_a, in_=rec_ta)
                # batch-reciprocal the denominators across 128 lanes
                if b == B - 1:
                    # only tcb3's 2 slots remain; the rest ran after tcb2
                    den_t = recpool.tile([128, 2 * TCH // 128], dt.bfloat16, tag="dent")
                    nc.sync.dma_start(out=den_t, in_=den_b[D:D + 1, 6 * TCH:8 * TCH])
                    rec_t = recpool.tile([128, 2 * TCH // 128], dt.bfloat16, tag="rect")
                    with nc.allow_low_precision(reason="bf16 softmax denom recip is plenty at rel-err 2e-2"):
                        nc.vector.reciprocal(out=rec_t, in_=den_t)
                    rec_z = recpool.tile([1, 2 * TCH], dt.bfloat16, tag="recall")
                    nc.sync.dma_start(out=rec_z, in_=rec_t)
                    pend = (b, att_un, (rec_a, rec_z))
                else:
                    den_t = recpool.tile([128, SLOTS * TCH // 128], dt.bfloat16, tag="dent")
                    sc_d = nc.sync.dma_start(out=den_t, in_=den_b[D:D + 1, :])
                    rec_t = recpool.tile([128, SLOTS * TCH // 128], dt.bfloat16, tag="rect")
                    with nc.allow_low_precision(reason="bf16 softmax denom recip is plenty at rel-err 2e-2"):
                        nc.vector.reciprocal(out=rec_t, in_=den_t)
                    rec_all = recpool.tile([1, SLOTS * TCH], dt.bfloat16, tag="recall")
                    ga_d = nc.sync.dma_start(out=rec_all, in_=rec_t)
                    pend = (b, att_un, rec_all)
            finalize(*pend)

            # ---- phase C: output projection on this core's token shards
            for b in range(B):
                rcv = rcv_tiles[b]
                if debug and b == 0:
                    nc.scalar.dma_start(out=dbg["rcv"][:, :, :], in_=rcv)
                for tt in range(TS // TT):
                    for c2 in range(CO // 512):
                        ps = psA.tile([128, TCH], dt.float32, tag="mm")
                        for j in range(N_CORES):
                            nc.tensor.matmul(
                                ps[0:TT, 0:512],
                                lhsT=rcv[:, j, tt * TT:(tt + 1) * TT],
                                rhs=wo_sb[:, j, c2 * 512:(c2 + 1) * 512],
                                start=(j == 0), stop=(j == N_CORES - 1),
                            )
                        osb = outpool.tile([TT, 512], dt.bfloat16, tag="osb")
                        nc.vector.tensor_add(
                            out=osb, in0=ps[0:TT, 0:512],
                            in1=bias_bc[0:TT, c2 * 512:(c2 + 1) * 512],
                        )
                        nc.scalar.dma_start(
                            out=out_d[b, tt * TT:(tt + 1) * TT, c2 * 512:(c2 + 1) * 512],
                            in_=osb,
                        )

    nc.finalize()
    return nc


def prep_inputs(x, Wq, Wk, Wv, Wo, bo):
    """Host-side shard/layout prep. Returns in_maps for the 8 cores."""
    B, T, C = x.shape
    H = Wq.shape[0]
    CK = C // 128
    SBB = TCH // 128

    x = np.asarray(x, dtype=np.float32)
    xt = np.ascontiguousarray(x.reshape(B * T, C).T.astype(BF16))  # [C, B*T]
    # [128, NCH, CK, TCH]: each token chunk is contiguous per partition, so
    # a chunk DMA is 128 x 8KB descriptors instead of 1024 x 1KB.
    NCH = (B * T) // TCH
    xt = np.ascontiguousarray(
        xt.reshape(CK, 128, NCH, TCH).transpose(1, 2, 0, 3)
    )

    CO = Wo.shape[1]
    wo_h = np.ascontiguousarray(
        np.asarray(Wo, np.float32).astype(BF16).reshape(N_CORES, 128, CO).transpose(1, 0, 2)
    )
    bo_h = np.asarray(bo, np.float32).astype(BF16).reshape(1, CO)

    p = np.arange(128)[:, None, None]
    j = np.arange(SBB)[None, :, None]
    t = np.arange(TCH)[None, None, :]
    mask_h = (t >= p + j * 128).astype(BF16)

    FP8 = ml_dtypes.float8_e4m3fn
    x8 = np.ascontiguousarray(xt.astype(np.float32)).astype(FP8)

    in_maps = []
    for m in range(N_CORES):
        maps = {"xt": xt, "x8": x8, "wo": wo_h, "bo": bo_h, "mask": mask_h}
        for name, W in (("wq", Wq), ("wk", Wk), ("wv", Wv)):
            Ws = np.concatenate(
                [np.asarray(W[HPC * m + i], np.float32) for i in range(HPC)], axis=1
            )  # [C, F]
            if name in ("wq", "wk"):
                # x32 puts the ~0.02-scale weights into e4m3 normal range;
                # the kernel divides the exp scale by 32*32 to compensate
                maps[name] = np.ascontiguousarray(
                    (Ws * 32.0).astype(FP8).reshape(CK, 128, F).transpose(1, 0, 2)
                )
            else:
                maps[name] = np.ascontiguousarray(
                    Ws.astype(BF16).reshape(CK, 128, F).transpose(1, 0, 2)
                )
        in_maps.append(maps)
    return in_maps


_NC_CACHE = {}


def _get_nc(B, T, C):
    key = (B, T, C)
    if key not in _NC_CACHE:
        _NC_CACHE[key] = build_nc(B, T, C)
    return _NC_CACHE[key]


def kernel(x, Wq, Wk, Wv, Wo, bo, _trace=False):
    x = np.asarray(x)
    B, T, C = x.shape
    nc = _get_nc(B, T, C)
    in_maps = prep_inputs(x, Wq, Wk, Wv, Wo, bo)
    res = run_bass_kernel_spmd(
        nc, in_maps, core_ids=list(range(N_CORES)), trace=_trace
    )
    TS = T // N_CORES
    CO = np.asarray(Wo).shape[1]
    out = np.empty((B, T, CO), dtype=np.float32)
    for m in range(N_CORES):
        out[:, m * TS:(m + 1) * TS, :] = res.results[m]["out"]
    if _trace:
        kernel.last_result = res
    return out



# revision 48
# speedup vs baseline: 1.1653x; 1.1653x over previous
"""Trainium2 Bass kernel for nn_MultiHeadAttention_30846455119878.

8-core strategy:
  - Attention phase is head-sharded: core m owns heads {2m, 2m+1}. Every core
    computes q/k/v projections for its 2 heads over all B*T tokens, then causal
    softmax attention per (batch, head).
  - The output projection contracts over ALL heads, so instead of an expensive
    AllReduce of [B,T,C] partials, each batch's attention output features
    ([128 feats x T]) are exchanged with a small AllToAll that re-shards from
    heads -> tokens. Each core then computes the full output projection for its
    1/8 token slice (contraction over all 1024 features) plus bias, locally.
  - Host side: x is passed pre-transposed as x^T [C, B*T] in bf16 (fp32 can't
    DMA-transpose on TRN2); all matmuls run bf16 x bf16 -> fp32 PSUM.

Layouts (per core):
  xt   [128, NCH, CK, TCH] bf16  xt[p, ch, o, t] = x[ch*512+t, o*128+p] (chunk-contiguous)
  x8   same layout, fp8 e4m3 (feeds q/k DoubleRow projections)
  wq/wk [128, CK, 128] fp8 e4m3, host-scaled x32; wv [128, CK, 128] bf16
  wo   [128, 8, C] bf16     wo[p, j, c] = Wo[j*128+p, c]
  bo   [1, C] bf16
  mask [128, 4, 512] bf16   mask[p, j, t] = 1 if t >= p + j*128 else 0  (causal diag blocks)
  out  [B, T/8, C] bf16     core m holds tokens [m*T/8, (m+1)*T/8) of every batch

Performance state (HW, 8 axon trn2 cores): 441,873 / 446,277 ns after a 3-min
cool-down, rel err 6.58e-3. THERMAL NOISE DOMINATES A/B TESTS: the SW/GPIO
throttlers (200us loop, ham type-1 k4/n8 = half clock) react to recent board
power; back-to-back benchmark runs degrade to 470-520us with type-1 active
300us+. Benchmark protocol: sleep 180 first, trust only same-temperature
pairs. 2026-08-10 session wins (all kept in this file):
  - xt prefetched a full batch ahead (issue after phase A of prev batch);
    per-chunk-contiguous DRAM layout [128, NCH, CK, TCH]. The old just-in-time
    fetch starved 20-45us during a2a windows and each PE stall re-throttled
    the clock (one 28us gap cost ~40us extra of half-clock).
  - finalize (rb-broadcast + normalize + staging + cc) DEFERRED into the next
    batch's phase-A window: kills all 4 batch-boundary PE gaps (7.2us each,
    the den->recip->rb chain), and the a2as collapse 47->15-23us because
    staging rides the quiet phase-A DMA window.
  - q/k projections in fp8 e4m3 DoubleRow (wq/wk host-scaled x32, exp scale
    /1024): -16k PE cols/batch, rel err 4.0e-3 -> 6.6e-3. WORKS on HW.
  - dummy 16-elem AllToAll at boot: absorbs the 11.5us cc first-trigger
    latency AND pre-aligns cores (real a2as then ~20us).
  - rcv DMA on gpsimd directly after its cc (the documented-safe spot);
    rcvpool bufs=4 REQUIRED (bufs=2 deadlocks cc3 behind phase-C-b0 WAR).
  - bias folded into a boot-time PE broadcast + DVE tensor_add in phase C
    (PSUM input); out/osb bf16 (host upcasts); staging merged 16->2 DMAs/batch
    via DRAM-side rearrange; den tiles bf16.
FAILED this session (do not retry naively):
  - fp8 DoubleRow SCORES: needs q/k in [32,2,h,T] -> 32-partition PSUM->SBUF
    copies; narrow copies are free-dim-bound (cost = full-width) on DVE AND
    ScalarE -> +60-100us whichever engine takes them. Cross-quadrant
    32-partition copies themselves ARE correct on both engines (HW-verified).
  - v projection in fp8 e4m3: rel err 2.95e-2 > 2e-2 gate (v errors pass
    linearly to the output; softmax absorbs q/k errors ~10x better).
  - mask in fp8: DVE tensor_mul bf16xfp8 is ~1.6x slower per element.
  - phase C accumulators on psB: 470us vs 445 (single warm run, weak signal).
  - bulk chunk DMAs on the gpsimd queue: software-driven, ~25us for 512KB
    under boot contention (vs ~2us hardware queues) -> two new 7us PE gaps.
    gpsimd DMAs are fine ONLY for small/slack loads (wv, mask, rcv).
  - boot xt chunks split across scalar/sync: neutral (boot is ~8us DMA
    cold-start dominated, not queue-depth dominated).
PE is 90-97% busy (interval-merged) inside every batch window at the granted
clock -> cross-batch interleave of phase A into phase B has ~nothing to win;
per-tcb recip for the last batch is zero-sum (rb MMs are PE-serial either
way). Tail a2a wait (~22-47us) has no deferrable work left to hide it.
Current trace shape (cool run): batches ~80us (PE-column-bound at whatever
clock the throttler grants; ScalarE exp 50us/batch paces phase B only when
fully warm), boot ~12us, tail ~45us = trigger3 + a2a3 20us + phase C b3 12us
+ drain 11us. PE cols/batch ~107k = 44.6us floor at 2.4GHz.

Tuning results (HW-measured, for future iteration):
  - Batched 128-lane reciprocal via SBUF->SBUF reshape DMA: 579 -> 468us. KEEP.
  - Diag-block column skip (c0 slicing): 613 -> 579us. KEEP.
  - v computed directly in [s,d] layout (no DMA-transpose round trip): KEEP.
  - psA/psB = 5/3 best; 6/2 -> 525us; 4/4 untested alone.
  - qk/v1 bufs 3, epool 8, rb-copy on DVE: all within +-4us run noise.
  - Per-half AllToAll split (8 collectives): 564us — entry-barrier floor dominates.
  - Outproj inlined into the b-loop after each collective: 451us but WRONG results
    (rel 0.86) — HW-only ordering hazard, unresolved; sim passes. Worth debugging
    with dbg taps on rcv: the ~17us speedup is real if made sound.
  - reciprocal_approx_fast / gpsimd partition_broadcast: BROKEN in this runtime
    (custom DVE/GPSIMD library ops garbage through the bass2jax compile path).
  - XBAR dma_start_transpose: silently shears with strided src / unaligned dst.
  - This walrus build: max 1 sync-wait per instruction -> must use bacc.Bacc.
  - bf16 AllToAll + TensorE tile_position row-packing: verified correct on HW.

2026-08-09 session (all HW-measured, 2+ runs each) - failed restructures; the
pipeline is a tight ScalarE<->PE equilibrium (both ~97% busy per 86us batch);
ANY extra dependency injected inside the batch loop breaks HAM warmth (PE
drops to 1.2GHz; throttle_active 90us -> 500us) and costs +80..120us:
  - v as [F,T] proj + PE-transpose to [s,d]: -20us PE on paper, measured +85us
    (DVE<->PE ping-pong in phase A delays v1, stalls phase B).
  - Per-tcb normalize+staging: collectives drop to 9-16us (vs 26-44) but wall
    +115us (normalize chain mid-phase-B breaks the score->exp->att pipeline).
  - Exp batched to [128,1024]/[128,2048] ACTs (ScalarE 58->40us/batch): forces
    PSUM score bufs 2 (vs 5) -> scores wait ACT latency every group -> micro-
    stalls -> cold PE. Net +80us. 8 PSUM banks cannot hold wider score groups
    plus 2 att banks plus proj/rb rotation; tag-sharing couples WARs, worse.
  - Boot PE-warmup spin (40 MMs): neutral. Tail keep-warm spin: +5us.
  - collective_compute BLOCKS its engine queue until completion; Tile REORDERS
    engine streams (a DMA sync-dep'd on a collective was hoisted mid-batch on
    the Scalar queue, head-of-line blocking ACTs 35us). Anything depending on
    a collective must ride gpsimd AFTER its collective or sit at program end.
  - SBUF-side AP rearrange("p (j t) -> j p t") in a DMA reads garbage -> NaN.
  - GPSIMD cannot touch PSUM; DVE TensorTensor allows only ONE PSUM input;
    engine PSUM reads must start at 32-aligned partitions; PE transpose output
    dtype must equal input dtype (bf16 PSUM tile).
  - A2A duration is mostly entry skew: boot-stagger across the 8 axon cores
    (init barrier 38-102us observed) shifts everything; benchmark twice.
"""

import sys

if "/opt/trn_rl_repo" not in sys.path:
    sys.path.insert(0, "/opt/trn_rl_repo")

import numpy as np
import ml_dtypes

import concourse.bass as bass
import concourse.tile as tile
from concourse import bacc, mybir
from concourse.bass_utils import run_bass_kernel_spmd
from concourse.tile_rust import add_dep_helper

BF16 = ml_dtypes.bfloat16

# Full problem dims
B_FULL, T_FULL, C_FULL, H_FULL, D_HEAD = 4, 2048, 1024, 16, 64
N_CORES = 8
HPC = H_FULL // N_CORES  # heads per core = 2
F = HPC * D_HEAD         # per-core attention feature rows = 128
TCH = 512                # query-chunk (free dim of score matmuls)
D = D_HEAD


def build_nc(B=B_FULL, T=T_FULL, C=C_FULL, debug=False):
    """Build the SPMD Bass graph (same graph on all 8 cores)."""
    dt = mybir.dt
    CK = C // 128        # contraction chunks for projections
    NTC = T // TCH       # query chunks per sequence
    NSB = T // 128       # key blocks per sequence
    SBB = TCH // 128     # key blocks that overlap one query chunk diagonal = 4
    TS = (B * T) // (B * N_CORES)  # token shard per (batch, core) = T // 8
    CO = H_FULL * D_HEAD  # output feature dim (Wo cols) = 1024
    TT = 128 if TS % 128 == 0 else TS  # token tile for output projection
    # wq/wk are host-scaled by 32 each (fp8 range); undo the 1024x here
    scale = float(1.0 / np.sqrt(C)) / 1024.0

    nc = bacc.Bacc()
    NCH = (B * T) // TCH  # token chunks over the whole input
    xt_d = nc.declare_dram_parameter("xt", [128, NCH, CK, TCH], dt.bfloat16, isOutput=False)
    # q/k projections run fp8 DoubleRow (2 contraction chunks per pass, half
    # the PE columns). Host pre-scales Wq/Wk by 32 to dodge e4m3 subnormals;
    # undone in the exp scale. v stays bf16: e4m3 there costs 2.9e-2 rel err
    # (values are linearly exposed in the output).
    x8_d = nc.declare_dram_parameter("x8", [128, NCH, CK, TCH], dt.float8e4, isOutput=False)
    wq_d = nc.declare_dram_parameter("wq", [128, CK, F], dt.float8e4, isOutput=False)
    wk_d = nc.declare_dram_parameter("wk", [128, CK, F], dt.float8e4, isOutput=False)
    wv_d = nc.declare_dram_parameter("wv", [128, CK, F], dt.bfloat16, isOutput=False)
    wo_d = nc.declare_dram_parameter("wo", [128, N_CORES, CO], dt.bfloat16, isOutput=False)
    bo_d = nc.declare_dram_parameter("bo", [1, CO], dt.bfloat16, isOutput=False)
    mask_d = nc.declare_dram_parameter("mask", [128, SBB, TCH], dt.bfloat16, isOutput=False)
    out_d = nc.declare_dram_parameter("out", [B, TS, CO], dt.bfloat16, isOutput=True)

    dbg = {}
    if debug:
        dbg["attn"] = nc.declare_dram_parameter("dbg_attn", [D, T], dt.bfloat16, isOutput=True)
        dbg["v1"] = nc.declare_dram_parameter("dbg_v1", [128, T // 128, HPC, 80], dt.bfloat16, isOutput=True)
        dbg["rcv"] = nc.declare_dram_parameter("dbg_rcv", [128, N_CORES, TS], dt.bfloat16, isOutput=True)
    dumm_i = nc.dram_tensor("cc_dummy_i", [N_CORES, 1, 16], dt.bfloat16)
    dumm_o = nc.dram_tensor("cc_dummy_o", [N_CORES, 1, 16], dt.bfloat16)
    cc_in = [nc.dram_tensor(f"cc_in{b}", [N_CORES, F, TS], dt.bfloat16) for b in range(B)]
    cc_out = [nc.dram_tensor(f"cc_out{b}", [N_CORES, F, TS], dt.bfloat16) for b in range(B)]
    rg = [list(range(N_CORES))]

    with tile.TileContext(nc) as tc:
        from contextlib import ExitStack

        with ExitStack() as ctx:
            wpool = ctx.enter_context(tc.tile_pool(name="w", bufs=1))
            xpool = ctx.enter_context(tc.tile_pool(name="xt", bufs=4))
            x8pool = ctx.enter_context(tc.tile_pool(name="x8", bufs=4))
            qkpool = ctx.enter_context(tc.tile_pool(name="qk", bufs=2))
            v1pool = ctx.enter_context(tc.tile_pool(name="v1", bufs=2))
            epool = ctx.enter_context(tc.tile_pool(name="exp", bufs=6))
            apool = ctx.enter_context(tc.tile_pool(name="attn", bufs=4))
            recpool = ctx.enter_context(tc.tile_pool(name="rec", bufs=3))
            aupool = ctx.enter_context(tc.tile_pool(name="attu", bufs=2))
            denpool = ctx.enter_context(tc.tile_pool(name="den", bufs=2))
            rcvpool = ctx.enter_context(tc.tile_pool(name="rcv", bufs=4))
            outpool = ctx.enter_context(tc.tile_pool(name="osb", bufs=2))
            psA = ctx.enter_context(tc.tile_pool(name="psA", bufs=5, space="PSUM"))
            psB = ctx.enter_context(tc.tile_pool(name="psB", bufs=3, space="PSUM"))

            # resident constants
            wq_sb = wpool.tile([128, CK, F], dt.float8e4, tag="wq")
            wk_sb = wpool.tile([128, CK, F], dt.float8e4, tag="wk")
            wv_sb = wpool.tile([128, CK, F], dt.bfloat16, tag="wv")
            wo_sb = wpool.tile([128, N_CORES, CO], dt.bfloat16, tag="wo")
            bo_sb = wpool.tile([1, CO], dt.bfloat16, tag="bo")
            mask_sb = wpool.tile([128, SBB, TCH], dt.bfloat16, tag="mask")
            ones_sb = wpool.tile([D + 1, 128], dt.bfloat16, tag="ones")
            # boot order: first q/k weights (scalar) + first xt chunks (sync)
            # so the very first projection matmul can start ASAP; everything
            # else rides other queues behind them.
            nc.scalar.dma_start(out=wq_sb, in_=wq_d[:, :, :])
            nc.scalar.dma_start(out=wk_sb, in_=wk_d[:, :, :])

            def issue_xt(b, xt_engs=None):
                # xt_engs: per-chunk queue override for the bf16 copies (boot
                # only — spreads the cold-start load over all 3 DMA queues)
                tiles = []
                for tcb in range(NTC):
                    x8_sb = x8pool.tile([128, CK, TCH], dt.float8e4, tag="x8")
                    nc.sync.dma_start(out=x8_sb, in_=x8_d[:, b * NTC + tcb, :, :])
                    xt_sb = xpool.tile([128, CK, TCH], dt.bfloat16, tag="xt")
                    eng = (xt_engs[tcb] if xt_engs and xt_engs[tcb] else nc.sync)
                    eng.dma_start(out=xt_sb, in_=xt_d[:, b * NTC + tcb, :, :])
                    tiles.append((x8_sb, xt_sb))
                return tiles

            nc.gpsimd.dma_start(out=wv_sb, in_=wv_d[:, :, :])
            # NOTE: gpsimd DMAs are software-driven and slow (~25us for 512KB
            # under boot contention) — keep bulk chunk loads off gpsimd
            xt_cur = issue_xt(0)
            nc.gpsimd.dma_start(out=mask_sb, in_=mask_d[:, :, :])
            # wo only matters at phase C; ride sync behind batch-0 x8
            nc.sync.dma_start(out=wo_sb, in_=wo_d[:, :, :])
            nc.scalar.dma_start(out=bo_sb, in_=bo_d[:, :])
            nc.vector.memset(ones_sb, 1.0)
            bias_bc = wpool.tile([128, CO], dt.bfloat16, tag="biasbc")
            # tiny warmup collective: absorbs the ~11.5us first-trigger
            # latency of the cc subsystem before the first real AllToAll
            nc.gpsimd.collective_compute(
                "AllToAll", mybir.AluOpType.bypass, replica_groups=rg,
                ins=[dumm_i.ap().opt()], outs=[dumm_o.ap().opt()],
            )

            cc_insts = []
            SLOTS = NTC * HPC

            def finalize(fb, att_un_f, rec_all_f):
                # rb-broadcast + normalize + staging + collective for batch fb.
                # Deferred into batch fb+1's phase-A window so the den->recip
                # chain never leaves PE idle at the batch boundary. Head-major
                # so head 0's staging DMA overlaps head 1's normalize (matters
                # for the last batch, where this chain is the a2a tail path).
                stg_insts = []
                attn_f = [apool.tile([D, T], dt.bfloat16, tag="attn", name=f"attn_{fb}_{hh}") for hh in range(HPC)]
                for h in range(HPC):
                    for tcb in range(NTC):
                        slot = tcb * HPC + h
                        if isinstance(rec_all_f, tuple):
                            ra_f, rz_f = rec_all_f
                            rec_src = (ra_f[0:1, slot * TCH:(slot + 1) * TCH]
                                       if slot < 6 else
                                       rz_f[0:1, (slot - 6) * TCH:(slot - 5) * TCH])
                        else:
                            rec_src = rec_all_f[0:1, slot * TCH:(slot + 1) * TCH]
                        rb_ps = psA.tile([D, TCH], dt.float32, tag="mm")
                        nc.tensor.matmul(
                            rb_ps, lhsT=ones_sb[0:1, 0:D],
                            rhs=rec_src,
                            start=True, stop=True,
                        )
                        # one PSUM input is allowed on DVE tensor_tensor, so
                        # multiply straight out of PSUM (no rb copy)
                        nc.vector.tensor_mul(
                            attn_f[h][:, tcb * TCH:(tcb + 1) * TCH],
                            att_un_f[:, slot, :], rb_ps,
                        )
                    eng = nc.scalar if h == 0 else nc.sync
                    stg_insts.append(eng.dma_start(
                        out=cc_in[fb][:, h * D:(h + 1) * D, :].rearrange("j p t -> p j t"),
                        in_=attn_f[h],
                    ).ins)
                if debug and fb == 0:
                    nc.scalar.dma_start(out=dbg["attn"][:, :], in_=attn_f[0])
                cc = nc.gpsimd.collective_compute(
                    "AllToAll", mybir.AluOpType.bypass, replica_groups=rg,
                    ins=[cc_in[fb].ap().opt()], outs=[cc_out[fb].ap().opt()],
                )
                for s in stg_insts:
                    add_dep_helper(cc.ins, s, sync=True, reason="cc_in RAW")
                cc_insts.append(cc.ins)
                # gpsimd is blocked by the collective anyway, so a dependent
                # DMA here fires the instant the a2a lands (no head-of-line
                # risk on the busy queues)
                rcv = rcvpool.tile([128, N_CORES, TS], dt.bfloat16, tag="rcv")
                rcv_rd = nc.gpsimd.dma_start(
                    out=rcv, in_=cc_out[fb][:, :, :].rearrange("j p t -> p j t")
                )
                add_dep_helper(rcv_rd.ins, cc.ins, sync=True, reason="cc_out RAW")
                rcv_tiles.append(rcv)

            pend = None
            rcv_tiles = []
            for b in range(B):
                # ---- phase A: q/k projections ([d, t] layout) and v ([s, d] layout)
                qT = qkpool.tile([F, T], dt.bfloat16, tag="qT")
                kT = qkpool.tile([F, T], dt.bfloat16, tag="kT")
                v1 = v1pool.tile([128, NSB, HPC, 80], dt.bfloat16, tag="v1")
                nc.vector.memset(v1[:, :, :, D:D + 1], 1.0)
                for tcb in range(NTC):
                    x8_sb, xt_sb = xt_cur[tcb]
                    for w_sb, dstT in ((wq_sb, qT), (wk_sb, kT)):
                        ps = psA.tile([128, TCH], dt.float32, tag="mm")
                        for o2 in range(CK // 2):
                            nc.tensor.matmul(
                                ps,
                                lhsT=w_sb[:, 2 * o2:2 * o2 + 2, :],
                                rhs=x8_sb[:, 2 * o2:2 * o2 + 2, :],
                                start=(o2 == 0), stop=(o2 == CK // 2 - 1),
                                perf_mode=mybir.MatmulPerfMode.DoubleRow,
                            )
                        nc.vector.tensor_copy(
                            out=dstT[:, tcb * TCH:(tcb + 1) * TCH], in_=ps
                        )
                    # v directly in [s, d] layout: v[s, f] = sum_c x[s, c] Wv[c, f]
                    for ssub in range(SBB):
                        vps_full = psA.tile([128, TCH], dt.float32, tag="mm", name=f"vps_{b}_{tcb}_{ssub}")
                        vps = vps_full[:, 0:F]
                        for o in range(CK):
                            nc.tensor.matmul(
                                vps,
                                lhsT=xt_sb[:, o, ssub * 128:(ssub + 1) * 128],
                                rhs=wv_sb[:, o, :],
                                start=(o == 0), stop=(o == CK - 1),
                            )
                        st = tcb * SBB + ssub
                        for h in range(HPC):
                            nc.vector.tensor_copy(
                                out=v1[:, st, h, 0:D], in_=vps[:, h * D:(h + 1) * D]
                            )
                if b == 0:
                    # broadcast bo across 128 token rows once; phase C then
                    # adds it on DVE instead of spending a PE pass per tile
                    for c2 in range(CO // 512):
                        bps = psA.tile([128, TCH], dt.float32, tag="mm")
                        nc.tensor.matmul(
                            bps, lhsT=ones_sb[0:1, 0:128],
                            rhs=bo_sb[0:1, c2 * 512:(c2 + 1) * 512],
                            start=True, stop=True,
                        )
                        nc.vector.tensor_copy(
                            out=bias_bc[:, c2 * 512:(c2 + 1) * 512], in_=bps
                        )
                if debug and b == 0:
                    nc.scalar.dma_start(out=dbg["v1"][:, :, :, :], in_=v1)
                # prefetch the whole next batch's xt now: all of this batch's
                # chunks are consumed, so the WAR is clear, and phase B gives
                # ~60us of slack before the data is needed (the a2a traffic
                # stalls late just-in-time fetches for 20-30us otherwise).
                if b + 1 < B:
                    xt_cur = issue_xt(b + 1)
                if pend is not None:
                    finalize(*pend)
                    pend = None

                # ---- phase B: causal attention, both heads interleaved
                att_un = aupool.tile([D, SLOTS, TCH], dt.bfloat16, tag="attu")
                den_b = denpool.tile([D + 1, SLOTS * TCH], dt.bfloat16, tag="den")
                for tcb in range(NTC):
                    att_ps = [psB.tile([D + 1, TCH], dt.float32, tag="att", name=f"attps_{b}_{tcb}_{hh}") for hh in range(HPC)]
                    nsb = SBB * (tcb + 1)
                    for sb in range(nsb):
                        j0 = sb - SBB * tcb
                        # columns t < j0*128 of this (key-block, query-chunk) pair are
                        # fully causal-masked -> skip them in scores/exp/mask/att
                        c0 = j0 * 128 if j0 > 0 else 0
                        ets = []
                        for h in range(HPC):
                            s_ps = psA.tile([128, TCH], dt.float32, tag="mm")
                            nc.tensor.matmul(
                                s_ps[:, c0:TCH],
                                lhsT=kT[h * D:(h + 1) * D, sb * 128:(sb + 1) * 128],
                                rhs=qT[h * D:(h + 1) * D, tcb * TCH + c0:(tcb + 1) * TCH],
                                start=True, stop=True,
                                tile_position=(h * D, 0),
                            )
                            et = epool.tile([128, TCH], dt.bfloat16, tag="exp")
                            nc.scalar.activation(
                                out=et[:, c0:TCH], in_=s_ps[:, c0:TCH],
                                func=mybir.ActivationFunctionType.Exp, scale=scale,
                            )
                            if j0 >= 0:
                                nc.vector.tensor_mul(
                                    et[:, c0:TCH], et[:, c0:TCH],
                                    mask_sb[:, j0, c0:TCH],
                                )
                            ets.append(et)
                        for h in range(HPC):
                            nc.tensor.matmul(
                                att_ps[h][:, c0:TCH],
                                lhsT=v1[:, sb, h, 0:D + 1], rhs=ets[h][:, c0:TCH],
                                start=(sb == 0), stop=(sb == nsb - 1),
                            )
                    for h in range(HPC):
                        slot = tcb * HPC + h
                        # denominator first: it feeds the recip critical path
                        nc.vector.tensor_copy(
                            out=den_b[D:D + 1, slot * TCH:(slot + 1) * TCH],
                            in_=att_ps[h][D:D + 1, :],
                        )
                    for h in range(HPC):
                        slot = tcb * HPC + h
                        nc.vector.tensor_copy(out=att_un[:, slot, :], in_=att_ps[h][0:D, :])
                    # last batch: reciprocal for the first 3 query chunks can
                    # run now (DVE/DMA only), hiding its ~5us latency under
                    # tcb3's compute instead of exposing it pre-trigger
                    if b == B - 1 and tcb == NTC - 2:
                        den_ta = recpool.tile([128, 6 * TCH // 128], dt.bfloat16, tag="dent")
                        nc.sync.dma_start(out=den_ta, in_=den_b[D:D + 1, 0:6 * TCH])
                        rec_ta = recpool.tile([128, 6 * TCH // 128], dt.bfloat16, tag="rect")
                        with nc.allow_low_precision(reason="bf16 softmax denom recip is plenty at rel-err 2e-2"):
                            nc.vector.reciprocal(out=rec_ta, in_=den_ta)
                        rec_a = recpool.tile([1, 6 * TCH], dt.bfloat16, tag="recall")
                        nc.sync.dma_start(out=rec_a, in_=rec_ta)
                # batch-reciprocal the denominators across 128 lanes
                if b == B - 1:
                    # only tcb3's 2 slots remain; the rest ran after tcb2
                    den_t = recpool.tile([128, 2 * TCH // 128], dt.bfloat16, tag="dent")
                    nc.sync.dma_start(out=den_t, in_=den_b[D:D + 1, 6 * TCH:8 * TCH])
                    rec_t = recpool.tile([128, 2 * TCH // 128], dt.bfloat16, tag="rect")
                    with nc.allow_low_precision(reason="bf16 softmax denom recip is plenty at rel-err 2e-2"):
                        nc.vector.reciprocal(out=rec_t, in_=den_t)
                    rec_z = recpool.tile([1, 2 * TCH], dt.bfloat16, tag="recall")
                    nc.sync.dma_start(out=rec_z, in_=rec_t)
                    pend = (b, att_un, (rec_a, rec_z))
                else:
                    den_t = recpool.tile([128, SLOTS * TCH // 128], dt.bfloat16, tag="dent")
                    sc_d = nc.sync.dma_start(out=den_t, in_=den_b[D:D + 1, :])
                    rec_t = recpool.tile([128, SLOTS * TCH // 128], dt.bfloat16, tag="rect")
                    with nc.allow_low_precision(reason="bf16 softmax denom recip is plenty at rel-err 2e-2"):
                        nc.vector.reciprocal(out=rec_t, in_=den_t)
                    rec_all = recpool.tile([1, SLOTS * TCH], dt.bfloat16, tag="recall")
                    ga_d = nc.sync.dma_start(out=rec_all, in_=rec_t)
                    pend = (b, att_un, rec_all)
            finalize(*pend)

            # ---- phase C: output projection on this core's token shards
            for b in range(B):
                rcv = rcv_tiles[b]
                if debug and b == 0:
                    nc.scalar.dma_start(out=dbg["rcv"][:, :, :], in_=rcv)
                for tt in range(TS // TT):
                    for c2 in range(CO // 512):
                        ps = psA.tile([128, TCH], dt.float32, tag="mm")
                        for j in range(N_CORES):
                            nc.tensor.matmul(
                                ps[0:TT, 0:512],
                                lhsT=rcv[:, j, tt * TT:(tt + 1) * TT],
                                rhs=wo_sb[:, j, c2 * 512:(c2 + 1) * 512],
                                start=(j == 0), stop=(j == N_CORES - 1),
                            )
                        osb = outpool.tile([TT, 512], dt.bfloat16, tag="osb")
                        nc.vector.tensor_add(
                            out=osb, in0=ps[0:TT, 0:512],
                            in1=bias_bc[0:TT, c2 * 512:(c2 + 1) * 512],
                        )
                        nc.scalar.dma_start(
                            out=out_d[b, tt * TT:(tt + 1) * TT, c2 * 512:(c2 + 1) * 512],
                            in_=osb,
                        )

    nc.finalize()
    return nc


def prep_inputs(x, Wq, Wk, Wv, Wo, bo):
    """Host-side shard/layout prep. Returns in_maps for the 8 cores."""
    B, T, C = x.shape
    H = Wq.shape[0]
    CK = C // 128
    SBB = TCH // 128

    x = np.asarray(x, dtype=np.float32)
    xt = np.ascontiguousarray(x.reshape(B * T, C).T.astype(BF16))  # [C, B*T]
    # [128, NCH, CK, TCH]: each token chunk is contiguous per partition, so
    # a chunk DMA is 128 x 8KB descriptors instead of 1024 x 1KB.
    NCH = (B * T) // TCH
    xt = np.ascontiguousarray(
        xt.reshape(CK, 128, NCH, TCH).transpose(1, 2, 0, 3)
    )

    CO = Wo.shape[1]
    wo_h = np.ascontiguousarray(
        np.asarray(Wo, np.float32).astype(BF16).reshape(N_CORES, 128, CO).transpose(1, 0, 2)
    )
    bo_h = np.asarray(bo, np.float32).astype(BF16).reshape(1, CO)

    p = np.arange(128)[:, None, None]
    j = np.arange(SBB)[None, :, None]
    t = np.arange(TCH)[None, None, :]
    mask_h = (t >= p + j * 128).astype(BF16)

    FP8 = ml_dtypes.float8_e4m3fn
    x8 = np.ascontiguousarray(xt.astype(np.float32)).astype(FP8)

    in_maps = []
    for m in range(N_CORES):
        maps = {"xt": xt, "x8": x8, "wo": wo_h, "bo": bo_h, "mask": mask_h}
        for name, W in (("wq", Wq), ("wk", Wk), ("wv", Wv)):
            Ws = np.concatenate(
                [np.asarray(W[HPC * m + i], np.float32) for i in range(HPC)], axis=1
            )  # [C, F]
            if name in ("wq", "wk"):
                # x32 puts the ~0.02-scale weights into e4m3 normal range;
                # the kernel divides the exp scale by 32*32 to compensate
                maps[name] = np.ascontiguousarray(
                    (Ws * 32.0).astype(FP8).reshape(CK, 128, F).transpose(1, 0, 2)
                )
            else:
                maps[name] = np.ascontiguousarray(
                    Ws.astype(BF16).reshape(CK, 128, F).transpose(1, 0, 2)
                )
        in_maps.append(maps)
    return in_maps


_NC_CACHE = {}


def _get_nc(B, T, C):
    key = (B, T, C)
    if key not in _NC_CACHE:
        _NC_CACHE[key] = build_nc(B, T, C)
    return _NC_CACHE[key]


def kernel(x, Wq, Wk, Wv, Wo, bo, _trace=False):
    x = np.asarray(x)
    B, T, C = x.shape
    nc = _get_nc(B, T, C)
    in_maps = prep_inputs(x, Wq, Wk, Wv, Wo, bo)
    res = run_bass_kernel_spmd(
        nc, in_maps, core_ids=list(range(N_CORES)), trace=_trace
    )
    TS = T // N_CORES
    CO = np.asarray(Wo).shape[1]
    out = np.empty((B, T, CO), dtype=np.float32)
    for m in range(N_CORES):
        out[:, m * TS:(m + 1) * TS, :] = res.results[m]["out"]
    if _trace:
        kernel.last_result = res
    return out

